# revision 1
# baseline (speedup 1.0000x reference)
"""Trainium2 Bass kernel for a 2-layer PyG-style GraphTransformer.

Sharding: edges are partitioned by destination node (host-side sort by dst =
the edge partitioning step); destination nodes are range-sharded 8 ways.
Each core:
  - computes q/k/v/skip projections for its node shard (data-parallel),
  - AllGathers k||v so every core can gather by arbitrary src,
  - processes its edge shard: per 128-dst-node block, indirect-DMA gathers
    kv[src] and q[dst] rows, computes segment softmax (no max subtraction --
    logits are tiny by construction, exp is exact-safe) and aggregates
    messages with one-hot selection-matrix matmuls into PSUM,
  - repeats the same structure for the (2-channel) second layer,
  - writes its [Nc, 2] output shard; host concatenates shards.
"""

import math
import os
import sys

import numpy as np

for _p in ("/opt/trn_rl_repo", "/root/.axon_site/_ro/trn_rl_repo"):
    if os.path.isdir(_p) and _p not in sys.path:
        sys.path.insert(0, _p)

from contextlib import ExitStack

import concourse.bacc as bacc
import concourse.bass as bass
import concourse.mybir as mybir
import concourse.tile as tile
from concourse.bass_utils import run_bass_kernel_spmd
from concourse.masks import make_identity

F32 = mybir.dt.float32
I32 = mybir.dt.int32
ALU = mybir.AluOpType
ACTF = mybir.ActivationFunctionType

N_CORES = 8
P = 128


def _build_program(N, Nc, NBLK, T, heads, hid, outc, dbg=False):
    """Build the SPMD bass program (identical on all cores).

    N: total nodes; Nc: nodes per core (N == N_CORES * Nc);
    NBLK: ceil(Nc / 128); T: subtiles (of 128 edge slots) per node block;
    heads/hid: layer-1 heads and per-head channels; outc: layer-2 channels.
    """
    C = heads * hid  # 128 feature channels
    S = NBLK * T  # index columns per core
    NPAD = NBLK * P  # padded node count per core
    KV = 2 * C  # k||v row width
    KV2 = 2 * outc

    nc = bacc.Bacc(
        "TRN2",
        target_bir_lowering=False,
        debug=False,
        enable_asserts=False,
        num_devices=N_CORES,
    )

    # ---- external I/O -------------------------------------------------
    xT_d = nc.dram_tensor("xT", [P, NPAD], F32, kind="ExternalInput")
    W1_d = nc.dram_tensor("W1cat", [C, KV + C], F32, kind="ExternalInput")
    b1_d = nc.dram_tensor("b1cat", [P, KV + C], F32, kind="ExternalInput")
    Ws1_d = nc.dram_tensor("Ws1", [C, C], F32, kind="ExternalInput")
    bs1_d = nc.dram_tensor("bs1", [P, C], F32, kind="ExternalInput")
    W2_d = nc.dram_tensor("W2cat", [C, 4 * outc], F32, kind="ExternalInput")
    b2_d = nc.dram_tensor("b2cat", [P, 4 * outc], F32, kind="ExternalInput")
    iota_d = nc.dram_tensor("iota", [P, P], F32, kind="ExternalInput")
    srcI_d = nc.dram_tensor("srcIdx", [P, S], I32, kind="ExternalInput")
    qdstI_d = nc.dram_tensor("qdstIdx", [P, S], I32, kind="ExternalInput")
    dstL_d = nc.dram_tensor("dstL", [P, S], F32, kind="ExternalInput")
    out_d = nc.dram_tensor("out", [Nc, outc], F32, kind="ExternalOutput")
    if dbg:
        dbg_kvf = nc.dram_tensor("dbg_kvf", [N, 2 * heads * hid], F32, kind="ExternalOutput")
        dbg_qtab = nc.dram_tensor("dbg_qtab", [NBLK * P, heads * hid], F32, kind="ExternalOutput")
        dbg_h = nc.dram_tensor("dbg_h", [P, NBLK * P], F32, kind="ExternalOutput")
        dbg_kv2f = nc.dram_tensor("dbg_kv2f", [N, 2 * outc], F32, kind="ExternalOutput")
        dbg_kvg = nc.dram_tensor("dbg_kvg", [P, T * 2 * heads * hid], F32, kind="ExternalOutput")
        dbg_qg = nc.dram_tensor("dbg_qg", [P, T * heads * hid], F32, kind="ExternalOutput")
        dbg_es = nc.dram_tensor("dbg_es", [P, T * heads], F32, kind="ExternalOutput")
        dbg_A = nc.dram_tensor("dbg_A", [P, T * P], F32, kind="ExternalOutput")

    # ---- internal DRAM ------------------------------------------------
    kv_sh = nc.dram_tensor("kv_sh", [Nc, KV], F32)
    kv_full = nc.dram_tensor("kv_full", [N, KV], F32, addr_space="Shared")
    q_tab = nc.dram_tensor("q_tab", [NPAD, C], F32)
    kv2_sh = nc.dram_tensor("kv2_sh", [Nc, KV2], F32)
    kv2_full = nc.dram_tensor("kv2_full", [N, KV2], F32, addr_space="Shared")
    q2_tab = nc.dram_tensor("q2_tab", [NPAD, outc], F32)

    rg = [list(range(N_CORES))]

    with tile.TileContext(nc) as tc, ExitStack() as ctx:
        cp = ctx.enter_context(tc.tile_pool(name="const", bufs=1))

        # constants / resident tiles
        W1_sb = cp.tile([C, KV + C], F32)
        nc.sync.dma_start(W1_sb[:], W1_d[:, :])
        b1_sb = cp.tile([P, KV + C], F32)
        nc.sync.dma_start(b1_sb[:], b1_d[:, :])
        Ws1_sb = cp.tile([C, C], F32)
        nc.sync.dma_start(Ws1_sb[:], Ws1_d[:, :])
        bs1_sb = cp.tile([P, C], F32)
        nc.sync.dma_start(bs1_sb[:], bs1_d[:, :])
        W2_sb = cp.tile([C, 4 * outc], F32)
        nc.sync.dma_start(W2_sb[:], W2_d[:, :])
        b2_sb = cp.tile([P, 4 * outc], F32)
        nc.sync.dma_start(b2_sb[:], b2_d[:, :])
        iota_sb = cp.tile([P, P], F32)
        nc.sync.dma_start(iota_sb[:], iota_d[:, :])
        srcI_sb = cp.tile([P, S], I32)
        nc.sync.dma_start(srcI_sb[:], srcI_d[:, :])
        qdstI_sb = cp.tile([P, S], I32)
        nc.sync.dma_start(qdstI_sb[:], qdstI_d[:, :])
        dstL_sb = cp.tile([P, S], F32)
        nc.sync.dma_start(dstL_sb[:], dstL_d[:, :])
        ident_sb = cp.tile([P, P], F32)
        make_identity(nc, ident_sb[:])

        s_skip = cp.tile([P, NPAD], F32)
        h_sb = cp.tile([P, NPAD], F32)
        qs2_sb = cp.tile([P, NBLK * 4 * outc], F32)

        # ---- phase 1: layer-1 projections -----------------------------
        with (
            tc.tile_pool(name="p1", bufs=3) as p1,
            tc.tile_pool(name="p1ps", bufs=2, space="PSUM") as p1ps,
        ):
            for b in range(NBLK):
                rows = min(P, Nc - b * P)
                xt = p1.tile([P, P], F32)
                nc.sync.dma_start(xt[:], xT_d[:, bass.ts(b, P)])
                ps = p1ps.tile([P, KV + C], F32)
                nc.tensor.matmul(ps[:], lhsT=xt[:], rhs=W1_sb[:], start=True, stop=True)
                ps2 = p1ps.tile([P, C], F32)
                nc.tensor.matmul(
                    ps2[:], lhsT=xt[:], rhs=Ws1_sb[:], start=True, stop=True
                )
                kvq = p1.tile([P, KV + C], F32)
                nc.vector.tensor_tensor(kvq[:], ps[:], b1_sb[:], op=ALU.add)
                nc.vector.tensor_tensor(
                    s_skip[:, bass.ts(b, P)], ps2[:], bs1_sb[:], op=ALU.add
                )
                nc.sync.dma_start(
                    kv_sh[b * P : b * P + rows, :], kvq[:rows, 0:KV]
                )
                nc.sync.dma_start(
                    q_tab[b * P : (b + 1) * P, :], kvq[:, KV : KV + C]
                )

        nc.gpsimd.collective_compute(
            "AllGather",
            ALU.bypass,
            replica_groups=rg,
            ins=[kv_sh[:, :]],
            outs=[kv_full[:, :]],
        )

        # ---- phase 2: layer-1 edge aggregation ------------------------
        with (
            tc.tile_pool(name="p2", bufs=2) as p2,
            tc.tile_pool(name="p2s", bufs=3) as p2s,
            tc.tile_pool(name="p2ps", bufs=2, space="PSUM") as p2ps,
        ):
            for b in range(NBLK):
                cols = slice(b * T, (b + 1) * T)
                kvg = p2.tile([P, T, KV], F32)
                qg = p2.tile([P, T, C], F32)
                for t in range(T):
                    col = b * T + t
                    nc.gpsimd.indirect_dma_start(
                        out=kvg[:, t, :],
                        out_offset=None,
                        in_=kv_full[:, :],
                        in_offset=bass.IndirectOffsetOnAxis(
                            ap=srcI_sb[:, col : col + 1], axis=0
                        ),
                    )
                    nc.gpsimd.indirect_dma_start(
                        out=qg[:, t, :],
                        out_offset=None,
                        in_=q_tab[:, :],
                        in_offset=bass.IndirectOffsetOnAxis(
                            ap=qdstI_sb[:, col : col + 1], axis=0
                        ),
                    )
                # one-hot selection matrix A[e, t, n] = (dst_local == n)
                A = p2.tile([P, T, P], F32)
                nc.vector.tensor_tensor(
                    A[:],
                    iota_sb[:].rearrange("p (a n) -> p a n", a=1).to_broadcast(
                        [P, T, P]
                    ),
                    dstL_sb[:, cols].rearrange("p (t a) -> p t a", a=1).to_broadcast(
                        [P, T, P]
                    ),
                    op=ALU.is_equal,
                )
                # logits: qk product then per-head reduce
                nc.vector.tensor_tensor(
                    qg[:], qg[:], kvg[:, :, 0:C], op=ALU.mult
                )
                es = p2s.tile([P, T, heads], F32)
                nc.vector.tensor_reduce(
                    es[:],
                    qg[:].rearrange("p t (h c) -> p t h c", c=hid),
                    axis=mybir.AxisListType.X,
                    op=ALU.add,
                )
                rhs = p2.tile([P, T, C + heads], F32)
                nc.scalar.activation(rhs[:, :, C : C + heads], es[:], ACTF.Exp)
                nc.vector.tensor_tensor(
                    rhs[:, :, 0:C].rearrange("p t (h c) -> p t h c", c=hid),
                    kvg[:, :, C:KV].rearrange("p t (h c) -> p t h c", c=hid),
                    rhs[:, :, C : C + heads]
                    .rearrange("p t (h a) -> p t h a", a=1)
                    .to_broadcast([P, T, heads, hid]),
                    op=ALU.mult,
                )
                if dbg and b == 0:
                    nc.sync.dma_start(dbg_kvg[:, :], kvg[:].rearrange("p t k -> p (t k)"))
                    nc.sync.dma_start(dbg_qg[:, :], qg[:].rearrange("p t k -> p (t k)"))
                    nc.sync.dma_start(dbg_es[:, :], es[:].rearrange("p t h -> p (t h)"))
                    nc.sync.dma_start(dbg_A[:, :], A[:].rearrange("p t n -> p (t n)"))
                pso = p2ps.tile([P, C + heads], F32)
                for t in range(T):
                    nc.tensor.matmul(
                        pso[:],
                        lhsT=A[:, t, :],
                        rhs=rhs[:, t, :],
                        start=(t == 0),
                        stop=(t == T - 1),
                    )
                stmp = p2s.tile([P, heads], F32)
                nc.vector.tensor_scalar_add(stmp[:], pso[:, C : C + heads], 1e-16)
                srec = p2s.tile([P, heads], F32)
                nc.vector.reciprocal(srec[:], stmp[:])
                hat = p2s.tile([P, C], F32)
                nc.vector.tensor_tensor(
                    hat[:].rearrange("p (h c) -> p h c", c=hid),
                    pso[:, 0:C].rearrange("p (h c) -> p h c", c=hid),
                    srec[:].rearrange("p (h a) -> p h a", a=1).to_broadcast(
                        [P, heads, hid]
                    ),
                    op=ALU.mult,
                )
                nc.vector.tensor_tensor(
                    hat[:], hat[:], s_skip[:, bass.ts(b, P)], op=ALU.add
                )
                nc.scalar.activation(h_sb[:, bass.ts(b, P)], hat[:], ACTF.Relu)

        if dbg:
            nc.sync.dma_start(dbg_kvf[:, :], kv_full[:, :])
            nc.sync.dma_start(dbg_qtab[:, :], q_tab[:, :])
            nc.sync.dma_start(dbg_h[:, :], h_sb[:])

        # ---- phase 3: layer-2 projections -----------------------------
        with (
            tc.tile_pool(name="p3", bufs=3) as p3,
            tc.tile_pool(name="p3ps", bufs=2, space="PSUM") as p3ps,
        ):
            for b in range(NBLK):
                rows = min(P, Nc - b * P)
                psT = p3ps.tile([P, P], F32)
                nc.tensor.transpose(psT[:], h_sb[:, bass.ts(b, P)], ident_sb[:])
                hT = p3.tile([P, P], F32)
                nc.scalar.copy(hT[:], psT[:])
                ps8 = p3ps.tile([P, 4 * outc], F32)
                nc.tensor.matmul(
                    ps8[:], lhsT=hT[:], rhs=W2_sb[:], start=True, stop=True
                )
                qs = qs2_sb[:, b * 4 * outc : (b + 1) * 4 * outc]
                nc.vector.tensor_tensor(qs, ps8[:], b2_sb[:], op=ALU.add)
                nc.sync.dma_start(
                    kv2_sh[b * P : b * P + rows, :],
                    qs2_sb[:rows, b * 4 * outc : b * 4 * outc + KV2],
                )
                nc.sync.dma_start(
                    q2_tab[b * P : (b + 1) * P, :],
                    qs2_sb[:, b * 4 * outc + KV2 : b * 4 * outc + 3 * outc],
                )

        nc.gpsimd.collective_compute(
            "AllGather",
            ALU.bypass,
            replica_groups=rg,
            ins=[kv2_sh[:, :]],
            outs=[kv2_full[:, :]],
        )

        if dbg:
            nc.sync.dma_start(dbg_kv2f[:, :], kv2_full[:, :])

        # ---- phase 4: layer-2 edge aggregation ------------------------
        with (
            tc.tile_pool(name="p4", bufs=2) as p4,
            tc.tile_pool(name="p4s", bufs=3) as p4s,
            tc.tile_pool(name="p4ps", bufs=2, space="PSUM") as p4ps,
        ):
            for b in range(NBLK):
                rows = min(P, Nc - b * P)
                cols = slice(b * T, (b + 1) * T)
                kv2g = p4.tile([P, T, KV2], F32)
                q2g = p4.tile([P, T, outc], F32)
                for t in range(T):
                    col = b * T + t
                    nc.gpsimd.indirect_dma_start(
                        out=kv2g[:, t, :],
                        out_offset=None,
                        in_=kv2_full[:, :],
                        in_offset=bass.IndirectOffsetOnAxis(
                            ap=srcI_sb[:, col : col + 1], axis=0
                        ),
                    )
                    nc.gpsimd.indirect_dma_start(
                        out=q2g[:, t, :],
                        out_offset=None,
                        in_=q2_tab[:, :],
                        in_offset=bass.IndirectOffsetOnAxis(
                            ap=qdstI_sb[:, col : col + 1], axis=0
                        ),
                    )
                A2 = p4.tile([P, T, P], F32)
                nc.vector.tensor_tensor(
                    A2[:],
                    iota_sb[:].rearrange("p (a n) -> p a n", a=1).to_broadcast(
                        [P, T, P]
                    ),
                    dstL_sb[:, cols].rearrange("p (t a) -> p t a", a=1).to_broadcast(
                        [P, T, P]
                    ),
                    op=ALU.is_equal,
                )
                nc.vector.tensor_tensor(
                    q2g[:], q2g[:], kv2g[:, :, 0:outc], op=ALU.mult
                )
                es2 = p4s.tile([P, T], F32)
                nc.vector.tensor_reduce(
                    es2[:],
                    q2g[:],
                    axis=mybir.AxisListType.X,
                    op=ALU.add,
                )
                rhs2 = p4.tile([P, T, 1 + outc], F32)
                nc.scalar.activation(
                    rhs2[:, :, 0:1], es2[:].rearrange("p (t a) -> p t a", a=1), ACTF.Exp
                )
                nc.vector.tensor_tensor(
                    rhs2[:, :, 1 : 1 + outc],
                    kv2g[:, :, outc:KV2],
                    rhs2[:, :, 0:1].to_broadcast([P, T, outc]),
                    op=ALU.mult,
                )
                pso2 = p4ps.tile([P, 1 + outc], F32)
                for t in range(T):
                    nc.tensor.matmul(
                        pso2[:],
                        lhsT=A2[:, t, :],
                        rhs=rhs2[:, t, :],
                        start=(t == 0),
                        stop=(t == T - 1),
                    )
                st2 = p4s.tile([P, 1], F32)
                nc.vector.tensor_scalar_add(st2[:], pso2[:, 0:1], 1e-16)
                sr2 = p4s.tile([P, 1], F32)
                nc.vector.reciprocal(sr2[:], st2[:])
                o2 = p4s.tile([P, outc], F32)
                nc.vector.tensor_tensor(
                    o2[:],
                    pso2[:, 1 : 1 + outc],
                    sr2[:].to_broadcast([P, outc]),
                    op=ALU.mult,
                )
                nc.vector.tensor_tensor(
                    o2[:],
                    o2[:],
                    qs2_sb[:, b * 4 * outc + 3 * outc : (b + 1) * 4 * outc],
                    op=ALU.add,
                )
                nc.sync.dma_start(out_d[b * P : b * P + rows, :], o2[:rows, :])

    nc.finalize()
    return nc


def _prepare(inputs, n_cores=N_CORES):
    """Host-side sharding: sort edges by dst, build per-core slot schedules."""
    x = np.asarray(inputs["x"], dtype=np.float32)
    ei = np.asarray(inputs["edge_index"])
    N = x.shape[0]
    heads, hid = 4, 32
    C = heads * hid
    outc = np.asarray(inputs["Wq2"]).shape[1]

    assert N % n_cores == 0, "node count must divide evenly across cores"
    Nc = N // n_cores
    NBLK = math.ceil(Nc / P)
    NPAD = NBLK * P

    src = ei[0].astype(np.int64)
    dst = ei[1].astype(np.int64)
    order = np.argsort(dst, kind="stable")
    ds = dst[order]
    ss = src[order]

    # per-(core, block) edge counts
    blk = ds // P  # global 128-node block id (n_cores*NBLK total... Nc%128 ok)
    # recompute as core-local block: core = ds // Nc ; local block = (ds - core*Nc)//P
    core = ds // Nc
    lblk = (ds - core * Nc) // P
    gb = core * NBLK + lblk
    counts = np.bincount(gb, minlength=n_cores * NBLK)
    T = max(1, int(np.ceil(counts.max() / P)))
    S = NBLK * T

    srcI = np.zeros((n_cores, P, S), dtype=np.int32)
    qdstI = np.zeros((n_cores, P, S), dtype=np.int32)
    dstL = np.full((n_cores, P, S), -1.0, dtype=np.float32)

    # block run boundaries in the sorted edge list
    starts = np.zeros(n_cores * NBLK + 1, dtype=np.int64)
    np.cumsum(counts, out=starts[1:])
    for c in range(n_cores):
        for b in range(NBLK):
            g = c * NBLK + b
            lo, hi = starts[g], starts[g + 1]
            k = hi - lo
            if k == 0:
                continue
            j = np.arange(k)
            col = b * T + j // P
            row = j % P
            srcI[c, row, col] = ss[lo:hi]
            qdstI[c, row, col] = ds[lo:hi] - c * Nc
            dstL[c, row, col] = (ds[lo:hi] - c * Nc - b * P).astype(np.float32)

    f32 = np.float32
    Wq1 = np.asarray(inputs["Wq1"], f32) / np.sqrt(np.float32(hid))
    bq1 = np.asarray(inputs["bq1"], f32) / np.sqrt(np.float32(hid))
    W1cat = np.concatenate(
        [np.asarray(inputs["Wk1"], f32), np.asarray(inputs["Wv1"], f32), Wq1], axis=1
    )
    b1cat = np.tile(
        np.concatenate([np.asarray(inputs["bk1"], f32), np.asarray(inputs["bv1"], f32), bq1])[None, :],
        (P, 1),
    )
    Ws1 = np.asarray(inputs["Ws1"], f32)
    bs1 = np.tile(np.asarray(inputs["bs1"], f32)[None, :], (P, 1))
    Wq2 = np.asarray(inputs["Wq2"], f32) / np.sqrt(np.float32(outc))
    bq2 = np.asarray(inputs["bq2"], f32) / np.sqrt(np.float32(outc))
    W2cat = np.concatenate(
        [
            np.asarray(inputs["Wk2"], f32),
            np.asarray(inputs["Wv2"], f32),
            Wq2,
            np.asarray(inputs["Ws2"], f32),
        ],
        axis=1,
    )
    b2cat = np.tile(
        np.concatenate(
            [np.asarray(inputs["bk2"], f32), np.asarray(inputs["bv2"], f32), bq2, np.asarray(inputs["bs2"], f32)]
        )[None, :],
        (P, 1),
    )
    iota = np.tile(np.arange(P, dtype=f32)[None, :], (P, 1))

    in_maps = []
    for c in range(n_cores):
        xT = np.zeros((C, NPAD), dtype=f32)
        xT[:, :Nc] = x[c * Nc : (c + 1) * Nc, :].T
        in_maps.append(
            {
                "xT": np.ascontiguousarray(xT),
                "W1cat": W1cat,
                "b1cat": b1cat,
                "Ws1": Ws1,
                "bs1": bs1,
                "W2cat": W2cat,
                "b2cat": b2cat,
                "iota": iota,
                "srcIdx": np.ascontiguousarray(srcI[c]),
                "qdstIdx": np.ascontiguousarray(qdstI[c]),
                "dstL": np.ascontiguousarray(dstL[c]),
            }
        )
    dims = dict(N=N, Nc=Nc, NBLK=NBLK, T=T, heads=heads, hid=hid, outc=outc)
    return in_maps, dims


_PROGRAM_CACHE = {}


def run(inputs, trace=False):
    in_maps, dims = _prepare(inputs)
    key = tuple(sorted(dims.items()))
    if key not in _PROGRAM_CACHE:
        _PROGRAM_CACHE[key] = _build_program(**dims)
    nc = _PROGRAM_CACHE[key]
    res = run_bass_kernel_spmd(
        nc, in_maps, core_ids=list(range(N_CORES)), trace=trace
    )
    Nc = dims["Nc"]
    out = np.concatenate([res.results[c]["out"] for c in range(N_CORES)], axis=0)
    return out.astype(np.float32), res


def kernel(**inputs):
    out, _ = run(inputs, trace=False)
    return out

